# revision 30
# baseline (speedup 1.0000x reference)
"""Causal Performer (FAVOR+) Trainium2 kernel, v2.

Sharding: 8 cores = 2 (batch) x 4 (head groups of 4 heads).  Each core
computes its 4 heads for one batch and returns a partial [4096, 2048]
output (its heads' contribution through w_o); the host sums the 4
partials per batch.

Key moves vs v1:
  - q/k head projections fused with the random-feature map on the host
    (qf = q @ (omega @ Wq_h).T), so on-chip contraction produces 8
    features per head (32 per core, padded to 128 rows at 32h offsets).
  - Causal scan chunked at 128 (not 512): per chunk, per head, one
    masked A^T matmul [128x128] + one intra numT matmul + one state
    matmul; state (Z | z) updated per chunk via one su matmul per head.
  - Denominator via a K-cumsum matmul chain shared by all 4 heads
    (stationary = seq-major normalized k features [128, 128], moving =
    causal mask) + block-diagonal reduction, instead of per-head
    ones-row matmul chains.
  - exp/square run once per tensor per block on [128, 512].
  - reciprocal_approx_fast for all reciprocals (values are >= eps).
  - x tiles double-buffered across blocks to keep the PE dense (HAM).
  - Output partials written bf16 (halves output DMA).

All matmuls bf16 with fp32 PSUM accumulation.
"""

import os
import numpy as np
import ml_dtypes

from concourse import bacc, mybir
import concourse.tile as tile
from concourse.bass import ts
from concourse.bass_utils import run_bass_kernel_spmd
from concourse.masks import make_identity

B, S, D = 2, 4096, 2048
H_PER = 4            # heads per core
DK = 128
NB = 8
SBLK = 512           # sequence block
NBLK = S // SBLK     # 8
NSUB = SBLK // 128   # 4 sub-chunks of 128
EPS = 1e-6

bf16 = mybir.dt.bfloat16
f32 = mybir.dt.float32

LAST_EXEC_TIME_NS = None
_CACHE = {}


def _build():
    nc = bacc.Bacc("TRN2", target_bir_lowering=False, debug=False)

    xq_d = nc.dram_tensor("xq", [D, S], bf16, kind="ExternalInput").ap()
    xk_d = nc.dram_tensor("xk", [D, S], bf16, kind="ExternalInput").ap()
    xv_d = nc.dram_tensor("xv", [D, S], bf16, kind="ExternalInput").ap()
    wqom_d = nc.dram_tensor("wqom", [D, 128], bf16, kind="ExternalInput").ap()
    wkom_d = nc.dram_tensor("wkom", [D, 128], bf16, kind="ExternalInput").ap()
    wv_d = nc.dram_tensor("wv", [D, 512], bf16, kind="ExternalInput").ap()
    wo_d = nc.dram_tensor("wo", [512, D], bf16, kind="ExternalInput").ap()
    mask_d = nc.dram_tensor("mask", [128, 128], f32, kind="ExternalInput").ap()
    maskb_d = nc.dram_tensor("maskb", [128, 512], bf16, kind="ExternalInput").ap()
    bd_d = nc.dram_tensor("bd", [128, 128], bf16, kind="ExternalInput").ap()
    part_d = nc.dram_tensor("part", [S, D], bf16, kind="ExternalOutput").ap()

    KC = D // 128    # 16 contraction chunks

    with tile.TileContext(nc) as tc:
        with tc.tile_pool(name="const", bufs=1) as const, \
             tc.tile_pool(name="wpool", bufs=1) as wpool, \
             tc.tile_pool(name="state", bufs=1) as state, \
             tc.tile_pool(name="xpool", bufs=2) as xpool, \
             tc.tile_pool(name="vpool", bufs=2) as vpool, \
             tc.tile_pool(name="featpool", bufs=2) as featpool, \
             tc.tile_pool(name="hfeat", bufs=8) as hfeat, \
             tc.tile_pool(name="atmpool", bufs=3) as atmpool, \
             tc.tile_pool(name="otpool", bufs=2) as otpool, \
             tc.tile_pool(name="osbpool", bufs=2) as osbpool, \
             tc.tile_pool(name="miscpool", bufs=2) as miscpool, \
             tc.tile_pool(name="psbig", bufs=4, space="PSUM") as psbig, \
             tc.tile_pool(name="psnum", bufs=1, space="PSUM") as psnum:

            ident = const.tile([128, 128], bf16, name="ident")
            make_identity(nc, ident)
            mask_sb = const.tile([128, 128], f32, name="mask_sb")
            nc.sync.dma_start(mask_sb[:], mask_d[:])
            maskb_sb = const.tile([128, 512], bf16, name="maskb_sb")
            nc.sync.dma_start(maskb_sb[:], maskb_d[:])
            bd_sb = const.tile([128, 128], bf16, name="bd_sb")
            nc.sync.dma_start(bd_sb[:], bd_d[:])
            ones_row = const.tile([97, 128], bf16, name="ones_row")
            nc.vector.memset(ones_row[:], 1.0)

            wqom_sb = wpool.tile([128, KC, 128], bf16, name="wqom_sb")
            nc.sync.dma_start(wqom_sb[:], wqom_d.rearrange("(c p) m -> p c m", p=128))
            wkom_sb = wpool.tile([128, KC, 128], bf16, name="wkom_sb")
            nc.sync.dma_start(wkom_sb[:], wkom_d.rearrange("(c p) m -> p c m", p=128))
            wv_sb = wpool.tile([128, KC, 512], bf16, name="wv_sb")
            nc.sync.dma_start(wv_sb[:], wv_d.rearrange("(c p) m -> p c m", p=128))
            wo_sb = wpool.tile([128, H_PER, D], bf16, name="wo_sb")
            nc.sync.dma_start(wo_sb[:], wo_d.rearrange("(c p) m -> p c m", p=128))

            # persistent scan state: Zsb rows at 32h (8 valid per head),
            # cols 0:128 = Z, col 128 = z (k' mass); zcol for K-cumsum.
            Zsb = state.tile([128, 132], f32, name="Zsb")
            nc.vector.memset(Zsb[:], 0.0)
            Zb16 = []
            for h in range(H_PER):
                zb = state.tile([NB, 132], bf16, name=f"Zb16_{h}")
                nc.vector.memset(zb[:], 0.0)
                Zb16.append(zb)
            zcol = []
            for h in range(H_PER):
                zc = state.tile([NB, 1], f32, name=f"zcol{h}")
                nc.vector.memset(zc[:], 0.0)
                zcol.append(zc)

            for blk in range(NBLK):
                s0 = blk * SBLK

                xq_sb = xpool.tile([128, KC, SBLK], bf16, name=f"xq{blk}", tag="xq")
                nc.gpsimd.dma_start(
                    xq_sb[:],
                    xq_d.rearrange("(c p) s -> p c s", p=128)[:, :, s0:s0 + SBLK])
                xk_sb = xpool.tile([128, KC, SBLK], bf16, name=f"xk{blk}", tag="xk")
                nc.gpsimd.dma_start(
                    xk_sb[:],
                    xk_d.rearrange("(c p) s -> p c s", p=128)[:, :, s0:s0 + SBLK])
                xv_sb = xpool.tile([128, KC, SBLK], bf16, name=f"xv{blk}", tag="xv")
                nc.gpsimd.dma_start(
                    xv_sb[:],
                    xv_d.rearrange("(c p) s -> p c s", p=128)[:, :, s0:s0 + SBLK])

                # ---- v projection: vha [s_sub(128), j, head, 132] (+ones col) ----
                vha = vpool.tile([128, NSUB, H_PER, 132], bf16, name=f"vha{blk}", tag="vha")
                for j in range(NSUB):
                    pp = psbig.tile([128, SBLK], f32, name=f"pv{blk}_{j}", tag="big")
                    for kc in range(KC):
                        nc.tensor.matmul(pp[:], xv_sb[:, kc, ts(j, 128)],
                                         wv_sb[:, kc, :],
                                         start=(kc == 0), stop=(kc == KC - 1))
                    nc.scalar.copy(vha[:, j, :, 0:128],
                                   pp.rearrange("p (h d) -> p h d", d=128))
                    nc.vector.memset(vha[:, j, :, 128:129], 1.0)

                # ---- fused q/k feature projections -> [128(pad), 512] ----
                qf_p = psbig.tile([128, SBLK], f32, name=f"qfp{blk}", tag="big")
                kf_p = psbig.tile([128, SBLK], f32, name=f"kfp{blk}", tag="big")
                for dst, wsb, xsb in ((qf_p, wqom_sb, xq_sb), (kf_p, wkom_sb, xk_sb)):
                    for kc in range(KC):
                        nc.tensor.matmul(dst[:], wsb[:, kc, :], xsb[:, kc, :],
                                         start=(kc == 0), stop=(kc == KC - 1))
                qsq = miscpool.tile([128, SBLK], f32, name=f"qsq{blk}", tag="sq")
                nc.scalar.square(qsq[:], qf_p[:])
                ksq = miscpool.tile([128, SBLK], f32, name=f"ksq{blk}", tag="sq")
                nc.scalar.square(ksq[:], kf_p[:])
                qfT = []
                kfT = []
                for h in range(H_PER):
                    qt = hfeat.tile([NB, SBLK], bf16, name=f"qfT{blk}_{h}", tag="qfT")
                    nc.scalar.activation(qt[:], qsq[32 * h:32 * h + NB, :],
                                         mybir.ActivationFunctionType.Exp, scale=-0.5)
                    qfT.append(qt)
                    # 32-row padded so transposes below cover full 32-bands
                    kt = hfeat.tile([32, SBLK], bf16, name=f"kfT{blk}_{h}", tag="kfT")
                    nc.vector.memset(kt[:], 0.0)
                    nc.scalar.activation(kt[0:NB, :], ksq[32 * h:32 * h + NB, :],
                                         mybir.ActivationFunctionType.Exp, scale=-0.5)
                    kfT.append(kt)

                # ---- k features seq-major + normalizer ----
                kfp2 = psbig.tile([128, NSUB, 128], bf16, name=f"kfp2{blk}", tag="big")
                for j in range(NSUB):
                    for h in range(H_PER):
                        nc.tensor.transpose(kfp2[:, j, 32 * h:32 * h + 32],
                                            kfT[h][0:32, ts(j, 128)], ident[0:32, 0:32])
                kfu = featpool.tile([128, NSUB, 128], bf16, name=f"kfu{blk}", tag="kfu")
                nc.vector.tensor_copy(kfu[:], kfp2[:])
                ksum = miscpool.tile([128, NSUB * H_PER], f32, name=f"ksum{blk}", tag="ksum")
                nc.vector.reduce_sum(
                    ksum.rearrange("p (j g) -> p j g", g=H_PER),
                    kfu.rearrange("p j (g n) -> p j g n", n=32)[:, :, :, 0:NB],
                    axis=mybir.AxisListType.X)
                nc.vector.tensor_scalar_add(ksum[:], ksum[:], EPS)
                krec = miscpool.tile([128, NSUB * H_PER], f32, name=f"krec{blk}", tag="krec")
                nc.vector.reciprocal(krec[:], ksum[:])
                kfn = featpool.tile([128, NSUB, 128], bf16, name=f"kfn{blk}", tag="kfn")
                for j in range(NSUB):
                    for h in range(H_PER):
                        nc.vector.tensor_scalar_mul(
                            kfn[:, j, 32 * h:32 * h + 32],
                            kfu[:, j, 32 * h:32 * h + 32],
                            krec[:, 4 * j + h:4 * j + h + 1])

                # ---- causal scan: masked A^T over the 512-block ----
                numT = []
                for h in range(H_PER):
                    numT.append(psnum.tile([128, SBLK], f32,
                                           name=f"numT{blk}_{h}", tag=f"numT{h}"))
                for h in range(H_PER):
                    atm = []
                    for i2 in range(NSUB):
                        n_i = SBLK - 128 * i2
                        at_p = psbig.tile([128, SBLK], f32,
                                          name=f"at{blk}_{h}_{i2}", tag="big")
                        nc.tensor.matmul(at_p[:, 0:n_i],
                                         kfT[h][0:NB, ts(i2, 128)],
                                         qfT[h][:, 128 * i2:SBLK],
                                         start=True, stop=True)
                        am = atmpool.tile([128, SBLK], bf16,
                                          name=f"am{blk}_{h}_{i2}", tag="atm")
                        # diagonal chunk: causal mask + k-normalizer (DVE)
                        nc.vector.scalar_tensor_tensor(
                            out=am[:, 0:128], in0=at_p[:, 0:128],
                            scalar=krec[:, 4 * i2 + h:4 * i2 + h + 1],
                            in1=mask_sb[:],
                            op0=mybir.AluOpType.mult, op1=mybir.AluOpType.mult)
                        # off-diagonal: k-normalizer only (Scalar engine)
                        if n_i > 128:
                            nc.scalar.mul(am[:, 128:n_i], at_p[:, 128:n_i],
                                          krec[:, 4 * i2 + h:4 * i2 + h + 1])
                        atm.append(am)
                    for i2 in range(NSUB):
                        nc.tensor.matmul(numT[h][:, 128 * i2:SBLK],
                                         vha[:, i2, h, 0:128],
                                         atm[i2][:, 0:SBLK - 128 * i2],
                                         start=(i2 == 0), stop=False)
                    nc.tensor.matmul(numT[h][:], Zb16[h][:, 0:128], qfT[h][:],
                                     start=False, stop=True)

                # state update (reads of Zb16 above precede these writes)
                su_p = psbig.tile([128, 512], f32, name=f"su{blk}", tag="big")
                for h in range(H_PER):
                    for i2 in range(NSUB):
                        nc.tensor.matmul(su_p[32 * h:32 * h + 32, 0:129],
                                         kfn[:, i2, 32 * h:32 * h + 32],
                                         vha[:, i2, h, 0:129],
                                         start=(i2 == 0), stop=(i2 == NSUB - 1),
                                         tile_position=(0, 32 * h))
                nc.vector.tensor_add(Zsb[:, 0:129], Zsb[:, 0:129], su_p[:, 0:129])
                for h in range(H_PER):
                    nc.vector.tensor_copy(Zb16[h][:, 0:129],
                                          Zsb[32 * h:32 * h + NB, 0:129])

                # ---- denominator via K-cumsum (all heads at once) ----
                kcum = psbig.tile([128, SBLK], f32, name=f"kcum{blk}", tag="big")
                for j in range(NSUB):
                    nc.tensor.matmul(kcum[:, 128 * j:SBLK],
                                     kfn[:, j, :], maskb_sb[:, 0:SBLK - 128 * j],
                                     start=(j == 0), stop=(j == NSUB - 1))
                prod = featpool.tile([128, SBLK], bf16, name=f"prod{blk}", tag="prod")
                nc.vector.memset(prod[:], 0.0)
                for h in range(H_PER):
                    nc.vector.scalar_tensor_tensor(
                        out=prod[32 * h:32 * h + NB, :],
                        in0=kcum[32 * h:32 * h + NB, :],
                        scalar=zcol[h][:, 0:1], in1=qfT[h][:],
                        op0=mybir.AluOpType.add, op1=mybir.AluOpType.mult)
                for h in range(H_PER):
                    nc.vector.tensor_add(zcol[h][:, 0:1], zcol[h][:, 0:1],
                                         kcum[32 * h:32 * h + NB, SBLK - 1:SBLK])
                den_p = psbig.tile([128, SBLK], f32, name=f"den{blk}", tag="big")
                nc.tensor.matmul(den_p[:], bd_sb[:], prod[:], start=True, stop=True)
                drr = miscpool.tile([97, SBLK], f32, name=f"drr{blk}", tag="drr")
                nc.vector.tensor_scalar_add(drr[:], den_p[0:97, :], EPS)
                drr2 = miscpool.tile([97, SBLK], f32, name=f"drr2{blk}", tag="drr2")
                nc.vector.reciprocal(drr2[:], drr[:])
                drb = miscpool.tile([97, SBLK], bf16, name=f"drb{blk}", tag="drb")
                nc.vector.tensor_copy(drb[:], drr2[:])

                outT = []
                for h in range(H_PER):
                    bc_p = psbig.tile([128, SBLK], f32, name=f"bcp{blk}_{h}", tag="big")
                    nc.tensor.matmul(bc_p[:], ones_row[32 * h:32 * h + 1, :],
                                     drb[32 * h:32 * h + 1, :],
                                     start=True, stop=True,
                                     tile_position=(32 * h, 0))
                    numc = miscpool.tile([128, SBLK], bf16, name=f"numc{blk}_{h}", tag="numc")
                    nc.vector.tensor_copy(numc[:], numT[h][:])
                    oT = otpool.tile([128, SBLK], bf16, name=f"oT{blk}_{h}", tag=f"outT{h}")
                    nc.vector.tensor_mul(oT[:], bc_p[:], numc[:])
                    outT.append(oT)

                # ---- output projection ----
                for j in range(NSUB):
                    osb = osbpool.tile([128, D], bf16, name=f"osb{blk}_{j}", tag="osb")
                    for c in range(4):
                        op = psbig.tile([128, 512], f32, name=f"op{blk}_{j}_{c}", tag="big")
                        for h in range(H_PER):
                            nc.tensor.matmul(op[:], outT[h][:, ts(j, 128)],
                                             wo_sb[:, h, ts(c, 512)],
                                             start=(h == 0), stop=(h == H_PER - 1))
                        if c % 2 == 0:
                            nc.scalar.copy(osb[:, ts(c, 512)], op[:])
                        else:
                            nc.vector.tensor_copy(osb[:, ts(c, 512)], op[:])
                    r0 = s0 + 128 * j
                    nc.sync.dma_start(part_d[r0:r0 + 128, :], osb[:])

    nc.compile()
    return nc


def _pad_feat(w):
    """[4, 8, D] head-feature weights -> [D, 128] with head h at rows 32h."""
    out = np.zeros((128, D), np.float32)
    for h in range(H_PER):
        out[32 * h:32 * h + NB] = w[h]
    return np.ascontiguousarray(out.T)


def _prep_inputs(q, k, v, w_q, w_k, w_v, w_o, omega):
    """Host-side sharding: returns in_maps for the 8 cores."""
    bf = ml_dtypes.bfloat16
    tri = np.triu(np.ones((128, 128), np.float32))   # mask[t, s] = t <= s
    mask = tri
    maskb = np.ones((128, 512), np.float32)
    maskb[:, :128] = tri
    bd = np.zeros((128, 128), np.float32)
    for h in range(H_PER):
        bd[32 * h:32 * h + NB, 32 * h] = 1.0

    xs = []
    for b in range(B):
        xs.append((np.ascontiguousarray(q[b].T).astype(bf),
                   np.ascontiguousarray(k[b].T).astype(bf),
                   np.ascontiguousarray(v[b].T).astype(bf)))

    # fused feature projections: per head, omega @ Wq_head  -> [8, 2048]
    wq_h = w_q.reshape(16, DK, D)                 # [head, dk, d_in]
    wk_h = w_k.reshape(16, DK, D)
    wqom = np.einsum('nd,hde->hne', omega, wq_h)  # [16, 8, D]
    wkom = np.einsum('nd,hde->hne', omega, wk_h)

    in_maps = []
    for core in range(8):
        b, g = divmod(core, 4)
        sl = slice(512 * g, 512 * (g + 1))
        hsl = slice(4 * g, 4 * (g + 1))
        xq, xk, xv = xs[b]
        in_maps.append({
            "xq": xq, "xk": xk, "xv": xv,
            "wqom": _pad_feat(wqom[hsl]).astype(bf),
            "wkom": _pad_feat(wkom[hsl]).astype(bf),
            "wv": np.ascontiguousarray(w_v[sl, :].T).astype(bf),
            "wo": np.ascontiguousarray(w_o[:, sl].T).astype(bf),
            "mask": mask,
            "maskb": maskb.astype(bf),
            "bd": bd.astype(bf),
        })
    return in_maps


def kernel(q, k, v, w_q, w_k, w_v, w_o, omega):
    global LAST_EXEC_TIME_NS
    q, k, v = np.asarray(q), np.asarray(k), np.asarray(v)
    w_q, w_k, w_v, w_o = (np.asarray(a) for a in (w_q, w_k, w_v, w_o))
    omega = np.asarray(omega)

    if "nc" not in _CACHE:
        _CACHE["nc"] = _build()
    nc = _CACHE["nc"]

    in_maps = _prep_inputs(q, k, v, w_q, w_k, w_v, w_o, omega)
    trace = bool(os.environ.get("BASS_KERNEL_TRACE"))
    res = run_bass_kernel_spmd(nc, in_maps, core_ids=list(range(8)), trace=trace)
    LAST_EXEC_TIME_NS = res.exec_time_ns

    out = np.zeros((B, S, D), np.float32)
    for core in range(8):
        b = core // 4
        out[b] += res.results[core]["part"].astype(np.float32)
    return out


# revision 31
# speedup vs baseline: 1.0958x; 1.0958x over previous
"""Causal Performer (FAVOR+) Trainium2 kernel, v2.

Sharding: 8 cores = 2 (batch) x 4 (head groups of 4 heads).  Each core
computes its 4 heads for one batch and returns a partial [4096, 2048]
output (its heads' contribution through w_o); the host sums the 4
partials per batch.

Key moves vs v1:
  - q/k head projections fused with the random-feature map on the host
    (qf = q @ (omega @ Wq_h).T), so on-chip contraction produces 8
    features per head (32 per core, padded to 128 rows at 32h offsets).
  - Causal scan chunked at 128 (not 512): per chunk, per head, one
    masked A^T matmul [128x128] + one intra numT matmul + one state
    matmul; state (Z | z) updated per chunk via one su matmul per head.
  - Denominator via a K-cumsum matmul chain shared by all 4 heads
    (stationary = seq-major normalized k features [128, 128], moving =
    causal mask) + block-diagonal reduction, instead of per-head
    ones-row matmul chains.
  - exp/square run once per tensor per block on [128, 512].
  - reciprocal_approx_fast for all reciprocals (values are >= eps).
  - x tiles double-buffered across blocks to keep the PE dense (HAM).
  - Output partials written bf16 (halves output DMA).

All matmuls bf16 with fp32 PSUM accumulation.
"""

import os
import numpy as np
import ml_dtypes

from concourse import bacc, mybir
import concourse.tile as tile
from concourse.bass import ts
from concourse.bass_utils import run_bass_kernel_spmd
from concourse.masks import make_identity

B, S, D = 2, 4096, 2048
H_PER = 4            # heads per core
DK = 128
NB = 8
SBLK = 512           # sequence block
NBLK = S // SBLK     # 8
NSUB = SBLK // 128   # 4 sub-chunks of 128
EPS = 1e-6

bf16 = mybir.dt.bfloat16
f32 = mybir.dt.float32

LAST_EXEC_TIME_NS = None
_CACHE = {}


def _build():
    nc = bacc.Bacc("TRN2", target_bir_lowering=False, debug=False)

    xq_d = nc.dram_tensor("xq", [D, S], bf16, kind="ExternalInput").ap()
    xk_d = nc.dram_tensor("xk", [D, S], bf16, kind="ExternalInput").ap()
    xv_d = nc.dram_tensor("xv", [D, S], bf16, kind="ExternalInput").ap()
    wqom_d = nc.dram_tensor("wqom", [D, 128], bf16, kind="ExternalInput").ap()
    wkom_d = nc.dram_tensor("wkom", [D, 128], bf16, kind="ExternalInput").ap()
    wv_d = nc.dram_tensor("wv", [D, 512], bf16, kind="ExternalInput").ap()
    wo_d = nc.dram_tensor("wo", [512, D], bf16, kind="ExternalInput").ap()
    mask_d = nc.dram_tensor("mask", [128, 128], f32, kind="ExternalInput").ap()
    maskb_d = nc.dram_tensor("maskb", [128, 512], bf16, kind="ExternalInput").ap()
    bd_d = nc.dram_tensor("bd", [128, 128], bf16, kind="ExternalInput").ap()
    part_d = nc.dram_tensor("part", [S, D], bf16, kind="ExternalOutput").ap()

    KC = D // 128    # 16 contraction chunks

    with tile.TileContext(nc) as tc:
        with tc.tile_pool(name="const", bufs=1) as const, \
             tc.tile_pool(name="wpool", bufs=1) as wpool, \
             tc.tile_pool(name="state", bufs=1) as state, \
             tc.tile_pool(name="xpool", bufs=2) as xpool, \
             tc.tile_pool(name="vpool", bufs=2) as vpool, \
             tc.tile_pool(name="featpool", bufs=2) as featpool, \
             tc.tile_pool(name="hfeat", bufs=8) as hfeat, \
             tc.tile_pool(name="atmpool", bufs=3) as atmpool, \
             tc.tile_pool(name="otpool", bufs=2) as otpool, \
             tc.tile_pool(name="osbpool", bufs=2) as osbpool, \
             tc.tile_pool(name="miscpool", bufs=2) as miscpool, \
             tc.tile_pool(name="psbig", bufs=4, space="PSUM") as psbig, \
             tc.tile_pool(name="psnum", bufs=1, space="PSUM") as psnum:

            ident = const.tile([128, 128], bf16, name="ident")
            make_identity(nc, ident)
            mask_sb = const.tile([128, 128], f32, name="mask_sb")
            nc.sync.dma_start(mask_sb[:], mask_d[:])
            maskb_sb = const.tile([128, 512], bf16, name="maskb_sb")
            nc.sync.dma_start(maskb_sb[:], maskb_d[:])
            bd_sb = const.tile([128, 128], bf16, name="bd_sb")
            nc.sync.dma_start(bd_sb[:], bd_d[:])
            ones_row = const.tile([97, 128], bf16, name="ones_row")
            nc.vector.memset(ones_row[:], 1.0)

            wqom_sb = wpool.tile([128, KC, 128], bf16, name="wqom_sb")
            nc.sync.dma_start(wqom_sb[:], wqom_d.rearrange("(c p) m -> p c m", p=128))
            wkom_sb = wpool.tile([128, KC, 128], bf16, name="wkom_sb")
            nc.sync.dma_start(wkom_sb[:], wkom_d.rearrange("(c p) m -> p c m", p=128))
            wv_sb = wpool.tile([128, KC, 512], bf16, name="wv_sb")
            nc.sync.dma_start(wv_sb[:], wv_d.rearrange("(c p) m -> p c m", p=128))
            wo_sb = wpool.tile([128, H_PER, D], bf16, name="wo_sb")
            nc.sync.dma_start(wo_sb[:], wo_d.rearrange("(c p) m -> p c m", p=128))

            # persistent scan state: Zsb rows at 32h (8 valid per head),
            # cols 0:128 = Z, col 128 = z (k' mass); zcol for K-cumsum.
            Zb16 = []
            for h in range(H_PER):
                zb = state.tile([NB, 132], bf16, name=f"Zb16_{h}")
                nc.vector.memset(zb[:], 0.0)
                Zb16.append(zb)
            zcol = []
            for h in range(H_PER):
                zc = state.tile([NB, 1], f32, name=f"zcol{h}")
                nc.vector.memset(zc[:], 0.0)
                zcol.append(zc)

            for blk in range(NBLK):
                s0 = blk * SBLK

                xv_sb = xpool.tile([128, KC, SBLK], bf16, name=f"xv{blk}", tag="xv")
                nc.gpsimd.dma_start(
                    xv_sb[:],
                    xv_d.rearrange("(c p) s -> p c s", p=128)[:, :, s0:s0 + SBLK])
                xq_sb = xpool.tile([128, KC, SBLK], bf16, name=f"xq{blk}", tag="xq")
                nc.gpsimd.dma_start(
                    xq_sb[:],
                    xq_d.rearrange("(c p) s -> p c s", p=128)[:, :, s0:s0 + SBLK])
                xk_sb = xpool.tile([128, KC, SBLK], bf16, name=f"xk{blk}", tag="xk")
                nc.gpsimd.dma_start(
                    xk_sb[:],
                    xk_d.rearrange("(c p) s -> p c s", p=128)[:, :, s0:s0 + SBLK])

                # ---- v projection: vha [s_sub(128), j, head, 132] (+ones col) ----
                vha = vpool.tile([128, NSUB, H_PER, 132], bf16, name=f"vha{blk}", tag="vha")
                for j in range(NSUB):
                    pp = psbig.tile([128, SBLK], f32, name=f"pv{blk}_{j}", tag="big")
                    for kc in range(KC):
                        nc.tensor.matmul(pp[:], xv_sb[:, kc, ts(j, 128)],
                                         wv_sb[:, kc, :],
                                         start=(kc == 0), stop=(kc == KC - 1))
                    nc.scalar.copy(vha[:, j, :, 0:128],
                                   pp.rearrange("p (h d) -> p h d", d=128))
                    nc.vector.memset(vha[:, j, :, 128:129], 1.0)

                # ---- fused q/k feature projections -> [128(pad), 512] ----
                qf_p = psbig.tile([128, SBLK], f32, name=f"qfp{blk}", tag="big")
                kf_p = psbig.tile([128, SBLK], f32, name=f"kfp{blk}", tag="big")
                for dst, wsb, xsb in ((qf_p, wqom_sb, xq_sb), (kf_p, wkom_sb, xk_sb)):
                    for kc in range(KC):
                        nc.tensor.matmul(dst[:], wsb[:, kc, :], xsb[:, kc, :],
                                         start=(kc == 0), stop=(kc == KC - 1))
                qsq = miscpool.tile([128, SBLK], f32, name=f"qsq{blk}", tag="sq")
                nc.scalar.square(qsq[:], qf_p[:])
                ksq = miscpool.tile([128, SBLK], f32, name=f"ksq{blk}", tag="sq")
                nc.scalar.square(ksq[:], kf_p[:])
                qfT = []
                kfT = []
                for h in range(H_PER):
                    qt = hfeat.tile([NB, SBLK], bf16, name=f"qfT{blk}_{h}", tag="qfT")
                    nc.scalar.activation(qt[:], qsq[32 * h:32 * h + NB, :],
                                         mybir.ActivationFunctionType.Exp, scale=-0.5)
                    qfT.append(qt)
                    # 32-row padded so transposes below cover full 32-bands
                    kt = hfeat.tile([32, SBLK], bf16, name=f"kfT{blk}_{h}", tag="kfT")
                    nc.vector.memset(kt[:], 0.0)
                    nc.scalar.activation(kt[0:NB, :], ksq[32 * h:32 * h + NB, :],
                                         mybir.ActivationFunctionType.Exp, scale=-0.5)
                    kfT.append(kt)

                # ---- k features seq-major + normalizer ----
                kfp2 = psbig.tile([128, NSUB, 128], bf16, name=f"kfp2{blk}", tag="big")
                for j in range(NSUB):
                    for h in range(H_PER):
                        nc.tensor.transpose(kfp2[:, j, 32 * h:32 * h + 32],
                                            kfT[h][0:32, ts(j, 128)], ident[0:32, 0:32])
                kfu = featpool.tile([128, NSUB, 128], bf16, name=f"kfu{blk}", tag="kfu")
                nc.vector.tensor_copy(kfu[:], kfp2[:])
                ksum = miscpool.tile([128, NSUB * H_PER], f32, name=f"ksum{blk}", tag="ksum")
                nc.vector.reduce_sum(
                    ksum.rearrange("p (j g) -> p j g", g=H_PER),
                    kfu.rearrange("p j (g n) -> p j g n", n=32)[:, :, :, 0:NB],
                    axis=mybir.AxisListType.X)
                nc.vector.tensor_scalar_add(ksum[:], ksum[:], EPS)
                krec = miscpool.tile([128, NSUB * H_PER], f32, name=f"krec{blk}", tag="krec")
                nc.vector.reciprocal_approx_fast(krec[:], ksum[:])
                kfn = featpool.tile([128, NSUB, 128], bf16, name=f"kfn{blk}", tag="kfn")
                for j in range(NSUB):
                    for h in range(H_PER):
                        nc.vector.tensor_scalar_mul(
                            kfn[:, j, 32 * h:32 * h + 32],
                            kfu[:, j, 32 * h:32 * h + 32],
                            krec[:, 4 * j + h:4 * j + h + 1])

                # ---- causal scan, 128-chunks ----
                numT = []
                for h in range(H_PER):
                    numT.append(psnum.tile([128, SBLK], f32,
                                           name=f"numT{blk}_{h}", tag=f"numT{h}"))
                for j in range(NSUB):
                    at4 = psbig.tile([128, H_PER, 128], f32,
                                     name=f"at{blk}_{j}", tag="big")
                    for h in range(H_PER):
                        nc.tensor.matmul(at4[:, h, :],
                                         kfT[h][0:NB, ts(j, 128)],
                                         qfT[h][:, ts(j, 128)],
                                         start=True, stop=True)
                    atm4 = atmpool.tile([128, H_PER, 128], bf16,
                                        name=f"atm{blk}_{j}", tag="atm")
                    for h in range(H_PER):
                        nc.vector.scalar_tensor_tensor(
                            out=atm4[:, h, :], in0=at4[:, h, :],
                            scalar=krec[:, 4 * j + h:4 * j + h + 1],
                            in1=mask_sb[:],
                            op0=mybir.AluOpType.mult, op1=mybir.AluOpType.mult)
                    for h in range(H_PER):
                        nc.tensor.matmul(numT[h][:, ts(j, 128)],
                                         vha[:, j, h, 0:128], atm4[:, h, :],
                                         start=True, stop=False)
                        nc.tensor.matmul(numT[h][:, ts(j, 128)],
                                         Zb16[h][:, 0:128],
                                         qfT[h][:, ts(j, 128)],
                                         start=False, stop=True)
                    # state update (reads of Zb16 above precede these writes)
                    # bank-pitched (512 f32) so 32h-partition slices stay bank-aligned
                    su_p = psbig.tile([128, 512], f32, name=f"su{blk}_{j}", tag="big")
                    for h in range(H_PER):
                        nc.tensor.matmul(su_p[32 * h:32 * h + 32, 0:129],
                                         kfn[:, j, 32 * h:32 * h + 32],
                                         vha[:, j, h, 0:129],
                                         start=True, stop=True,
                                         tile_position=(0, 32 * h))
                    for h in range(H_PER):
                        nc.vector.tensor_add(Zb16[h][:, 0:129], Zb16[h][:, 0:129],
                                             su_p[32 * h:32 * h + NB, 0:129])

                # ---- denominator via K-cumsum (all heads at once) ----
                kcum = psbig.tile([128, SBLK], f32, name=f"kcum{blk}", tag="big")
                for j in range(NSUB):
                    nc.tensor.matmul(kcum[:, 128 * j:SBLK],
                                     kfn[:, j, :], maskb_sb[:, 0:SBLK - 128 * j],
                                     start=(j == 0), stop=(j == NSUB - 1))
                prod = featpool.tile([128, SBLK], bf16, name=f"prod{blk}", tag="prod")
                nc.vector.memset(prod[:], 0.0)
                for h in range(H_PER):
                    nc.vector.scalar_tensor_tensor(
                        out=prod[32 * h:32 * h + NB, :],
                        in0=kcum[32 * h:32 * h + NB, :],
                        scalar=zcol[h][:, 0:1], in1=qfT[h][:],
                        op0=mybir.AluOpType.add, op1=mybir.AluOpType.mult)
                for h in range(H_PER):
                    nc.vector.tensor_add(zcol[h][:, 0:1], zcol[h][:, 0:1],
                                         kcum[32 * h:32 * h + NB, SBLK - 1:SBLK])
                den_p = psbig.tile([128, SBLK], f32, name=f"den{blk}", tag="big")
                nc.tensor.matmul(den_p[:], bd_sb[:], prod[:], start=True, stop=True)
                drr = miscpool.tile([97, SBLK], f32, name=f"drr{blk}", tag="drr")
                nc.vector.tensor_scalar_add(drr[:], den_p[0:97, :], EPS)
                drr2 = miscpool.tile([97, SBLK], f32, name=f"drr2{blk}", tag="drr2")
                nc.vector.reciprocal_approx_fast(drr2[:], drr[:])
                drb = miscpool.tile([97, SBLK], bf16, name=f"drb{blk}", tag="drb")
                nc.vector.tensor_copy(drb[:], drr2[:])

                outT = []
                for h in range(H_PER):
                    bc_p = psbig.tile([128, SBLK], f32, name=f"bcp{blk}_{h}", tag="big")
                    nc.tensor.matmul(bc_p[:], ones_row[32 * h:32 * h + 1, :],
                                     drb[32 * h:32 * h + 1, :],
                                     start=True, stop=True,
                                     tile_position=(32 * h, 0))
                    numc = miscpool.tile([128, SBLK], bf16, name=f"numc{blk}_{h}", tag="numc")
                    nc.vector.tensor_copy(numc[:], numT[h][:])
                    oT = otpool.tile([128, SBLK], bf16, name=f"oT{blk}_{h}", tag=f"outT{h}")
                    nc.vector.tensor_mul(oT[:], bc_p[:], numc[:])
                    outT.append(oT)

                # ---- output projection ----
                for j in range(NSUB):
                    osb = osbpool.tile([128, D], bf16, name=f"osb{blk}_{j}", tag="osb")
                    for c in range(4):
                        op = psbig.tile([128, 512], f32, name=f"op{blk}_{j}_{c}", tag="big")
                        for h in range(H_PER):
                            nc.tensor.matmul(op[:], outT[h][:, ts(j, 128)],
                                             wo_sb[:, h, ts(c, 512)],
                                             start=(h == 0), stop=(h == H_PER - 1))
                        if c != 3:
                            nc.scalar.copy(osb[:, ts(c, 512)], op[:])
                        else:
                            nc.vector.tensor_copy(osb[:, ts(c, 512)], op[:])
                    r0 = s0 + 128 * j
                    nc.sync.dma_start(part_d[r0:r0 + 128, :], osb[:])

    nc.compile()
    return nc


def _pad_feat(w):
    """[4, 8, D] head-feature weights -> [D, 128] with head h at rows 32h."""
    out = np.zeros((128, D), np.float32)
    for h in range(H_PER):
        out[32 * h:32 * h + NB] = w[h]
    return np.ascontiguousarray(out.T)


def _prep_inputs(q, k, v, w_q, w_k, w_v, w_o, omega):
    """Host-side sharding: returns in_maps for the 8 cores."""
    bf = ml_dtypes.bfloat16
    tri = np.triu(np.ones((128, 128), np.float32))   # mask[t, s] = t <= s
    mask = tri
    maskb = np.ones((128, 512), np.float32)
    maskb[:, :128] = tri
    bd = np.zeros((128, 128), np.float32)
    for h in range(H_PER):
        bd[32 * h:32 * h + NB, 32 * h] = 1.0

    xs = []
    for b in range(B):
        xs.append((np.ascontiguousarray(q[b].T).astype(bf),
                   np.ascontiguousarray(k[b].T).astype(bf),
                   np.ascontiguousarray(v[b].T).astype(bf)))

    # fused feature projections: per head, omega @ Wq_head  -> [8, 2048]
    wq_h = w_q.reshape(16, DK, D)                 # [head, dk, d_in]
    wk_h = w_k.reshape(16, DK, D)
    wqom = np.einsum('nd,hde->hne', omega, wq_h)  # [16, 8, D]
    wkom = np.einsum('nd,hde->hne', omega, wk_h)

    in_maps = []
    for core in range(8):
        b, g = divmod(core, 4)
        sl = slice(512 * g, 512 * (g + 1))
        hsl = slice(4 * g, 4 * (g + 1))
        xq, xk, xv = xs[b]
        in_maps.append({
            "xq": xq, "xk": xk, "xv": xv,
            "wqom": _pad_feat(wqom[hsl]).astype(bf),
            "wkom": _pad_feat(wkom[hsl]).astype(bf),
            "wv": np.ascontiguousarray(w_v[sl, :].T).astype(bf),
            "wo": np.ascontiguousarray(w_o[:, sl].T).astype(bf),
            "mask": mask,
            "maskb": maskb.astype(bf),
            "bd": bd.astype(bf),
        })
    return in_maps


def kernel(q, k, v, w_q, w_k, w_v, w_o, omega):
    global LAST_EXEC_TIME_NS
    q, k, v = np.asarray(q), np.asarray(k), np.asarray(v)
    w_q, w_k, w_v, w_o = (np.asarray(a) for a in (w_q, w_k, w_v, w_o))
    omega = np.asarray(omega)

    if "nc" not in _CACHE:
        _CACHE["nc"] = _build()
    nc = _CACHE["nc"]

    in_maps = _prep_inputs(q, k, v, w_q, w_k, w_v, w_o, omega)
    trace = bool(os.environ.get("BASS_KERNEL_TRACE"))
    res = run_bass_kernel_spmd(nc, in_maps, core_ids=list(range(8)), trace=trace)
    LAST_EXEC_TIME_NS = res.exec_time_ns

    out = np.zeros((B, S, D), np.float32)
    for core in range(8):
        b = core // 4
        out[b] += res.results[core]["part"].astype(np.float32)
    return out


# revision 32
# speedup vs baseline: 1.2293x; 1.1217x over previous
"""Causal Performer (FAVOR+) Trainium2 kernel, v2.

Sharding: 8 cores = 2 (batch) x 4 (head groups of 4 heads).  Each core
computes its 4 heads for one batch and returns a partial [4096, 2048]
output (its heads' contribution through w_o); the host sums the 4
partials per batch.

Key moves vs v1:
  - q/k head projections fused with the random-feature map on the host
    (qf = q @ (omega @ Wq_h).T), so on-chip contraction produces 8
    features per head (32 per core, padded to 128 rows at 32h offsets).
  - Causal scan chunked at 128 (not 512): per chunk, per head, one
    masked A^T matmul [128x128] + one intra numT matmul + one state
    matmul; state (Z | z) updated per chunk via one su matmul per head.
  - Denominator via a K-cumsum matmul chain shared by all 4 heads
    (stationary = seq-major normalized k features [128, 128], moving =
    causal mask) + block-diagonal reduction, instead of per-head
    ones-row matmul chains.
  - exp/square run once per tensor per block on [128, 512].
  - reciprocal_approx_fast for all reciprocals (values are >= eps).
  - x tiles double-buffered across blocks to keep the PE dense (HAM).
  - Output partials written bf16 (halves output DMA).

All matmuls bf16 with fp32 PSUM accumulation.
"""

import os
import numpy as np
import ml_dtypes

from concourse import bacc, mybir
import concourse.tile as tile
from concourse.bass import ts
from concourse.bass_utils import run_bass_kernel_spmd
from concourse.masks import make_identity

B, S, D = 2, 4096, 2048
H_PER = 4            # heads per core
DK = 128
NB = 8
SBLK = 512           # sequence block
NBLK = S // SBLK     # 8
NSUB = SBLK // 128   # 4 sub-chunks of 128
EPS = 1e-6

bf16 = mybir.dt.bfloat16
f32 = mybir.dt.float32

LAST_EXEC_TIME_NS = None
_CACHE = {}


def _build():
    nc = bacc.Bacc("TRN2", target_bir_lowering=False, debug=False)

    xq_d = nc.dram_tensor("xq", [D, S], bf16, kind="ExternalInput").ap()
    xk_d = nc.dram_tensor("xk", [D, S], bf16, kind="ExternalInput").ap()
    xv_d = nc.dram_tensor("xv", [D, S], bf16, kind="ExternalInput").ap()
    wqom_d = nc.dram_tensor("wqom", [D, 128], bf16, kind="ExternalInput").ap()
    wkom_d = nc.dram_tensor("wkom", [D, 128], bf16, kind="ExternalInput").ap()
    wv_d = nc.dram_tensor("wv", [D, 512], bf16, kind="ExternalInput").ap()
    wo_d = nc.dram_tensor("wo", [512, D], bf16, kind="ExternalInput").ap()
    mask_d = nc.dram_tensor("mask", [128, 128], f32, kind="ExternalInput").ap()
    maskb_d = nc.dram_tensor("maskb", [128, 512], bf16, kind="ExternalInput").ap()
    bd_d = nc.dram_tensor("bd", [128, 128], bf16, kind="ExternalInput").ap()
    part_d = nc.dram_tensor("part", [S, D], bf16, kind="ExternalOutput").ap()

    KC = D // 128    # 16 contraction chunks

    with tile.TileContext(nc) as tc:
        with tc.tile_pool(name="const", bufs=1) as const, \
             tc.tile_pool(name="wpool", bufs=1) as wpool, \
             tc.tile_pool(name="state", bufs=1) as state, \
             tc.tile_pool(name="xpool", bufs=2) as xpool, \
             tc.tile_pool(name="vpool", bufs=2) as vpool, \
             tc.tile_pool(name="featpool", bufs=2) as featpool, \
             tc.tile_pool(name="hfeat", bufs=8) as hfeat, \
             tc.tile_pool(name="atmpool", bufs=3) as atmpool, \
             tc.tile_pool(name="otpool", bufs=2) as otpool, \
             tc.tile_pool(name="osbpool", bufs=2) as osbpool, \
             tc.tile_pool(name="miscpool", bufs=2) as miscpool, \
             tc.tile_pool(name="psbig", bufs=2, space="PSUM") as psbig, \
             tc.tile_pool(name="psv", bufs=2, space="PSUM") as psv, \
             tc.tile_pool(name="psnum", bufs=1, space="PSUM") as psnum:

            ident = const.tile([128, 128], bf16, name="ident")
            make_identity(nc, ident)
            mask_sb = const.tile([128, 128], f32, name="mask_sb")
            nc.sync.dma_start(mask_sb[:], mask_d[:])
            maskb_sb = const.tile([128, 512], bf16, name="maskb_sb")
            nc.sync.dma_start(maskb_sb[:], maskb_d[:])
            bd_sb = const.tile([128, 128], bf16, name="bd_sb")
            nc.sync.dma_start(bd_sb[:], bd_d[:])
            ones_row = const.tile([97, 128], bf16, name="ones_row")
            nc.vector.memset(ones_row[:], 1.0)

            wqom_sb = wpool.tile([128, KC, 128], bf16, name="wqom_sb")
            nc.sync.dma_start(wqom_sb[:], wqom_d.rearrange("(c p) m -> p c m", p=128))
            wkom_sb = wpool.tile([128, KC, 128], bf16, name="wkom_sb")
            nc.sync.dma_start(wkom_sb[:], wkom_d.rearrange("(c p) m -> p c m", p=128))
            wv_sb = wpool.tile([128, KC, 512], bf16, name="wv_sb")
            nc.sync.dma_start(wv_sb[:], wv_d.rearrange("(c p) m -> p c m", p=128))
            wo_sb = wpool.tile([128, H_PER, D], bf16, name="wo_sb")
            nc.sync.dma_start(wo_sb[:], wo_d.rearrange("(c p) m -> p c m", p=128))

            # persistent scan state: Zsb rows at 32h (8 valid per head),
            # cols 0:128 = Z, col 128 = z (k' mass); zcol for K-cumsum.
            Zb16 = []
            for h in range(H_PER):
                zb = state.tile([NB, 132], bf16, name=f"Zb16_{h}")
                nc.vector.memset(zb[:], 0.0)
                Zb16.append(zb)
            zcol = []
            for h in range(H_PER):
                zc = state.tile([NB, 1], f32, name=f"zcol{h}")
                nc.vector.memset(zc[:], 0.0)
                zcol.append(zc)

            for blk in range(NBLK):
                s0 = blk * SBLK

                xv_sb = xpool.tile([128, KC, SBLK], bf16, name=f"xv{blk}", tag="xv")
                xq_sb = xpool.tile([128, KC, SBLK], bf16, name=f"xq{blk}", tag="xq")
                xk_sb = xpool.tile([128, KC, SBLK], bf16, name=f"xk{blk}", tag="xk")
                for xsb, xd in ((xv_sb, xv_d), (xq_sb, xq_d), (xk_sb, xk_d)):
                    xr = xd.rearrange("(c p) s -> p c s", p=128)
                    nc.gpsimd.dma_start(xsb[:, 0:KC // 2, :],
                                        xr[:, 0:KC // 2, s0:s0 + SBLK])
                    nc.gpsimd.dma_start(xsb[:, KC // 2:KC, :],
                                        xr[:, KC // 2:KC, s0:s0 + SBLK])

                # ---- v projection: vha [s_sub(128), j, head, 132] (+ones col) ----
                vha = vpool.tile([128, NSUB, H_PER, 132], bf16, name=f"vha{blk}", tag="vha")
                for j in range(NSUB):
                    pp = psv.tile([128, SBLK], f32, name=f"pv{blk}_{j}", tag="pv")
                    for kc in range(KC):
                        nc.tensor.matmul(pp[:], xv_sb[:, kc, ts(j, 128)],
                                         wv_sb[:, kc, :],
                                         start=(kc == 0), stop=(kc == KC - 1))
                    nc.scalar.copy(vha[:, j, :, 0:128],
                                   pp.rearrange("p (h d) -> p h d", d=128))
                    nc.vector.memset(vha[:, j, :, 128:129], 1.0)

                # ---- fused q/k feature projections -> [128(pad), 512] ----
                qf_p = psbig.tile([128, SBLK], f32, name=f"qfp{blk}", tag="big")
                kf_p = psbig.tile([128, SBLK], f32, name=f"kfp{blk}", tag="big")
                for dst, wsb, xsb in ((qf_p, wqom_sb, xq_sb), (kf_p, wkom_sb, xk_sb)):
                    for kc in range(KC):
                        nc.tensor.matmul(dst[:], wsb[:, kc, :], xsb[:, kc, :],
                                         start=(kc == 0), stop=(kc == KC - 1))
                qsq = miscpool.tile([128, SBLK], f32, name=f"qsq{blk}", tag="sq")
                nc.scalar.square(qsq[:], qf_p[:])
                ksq = miscpool.tile([128, SBLK], f32, name=f"ksq{blk}", tag="sq")
                nc.scalar.square(ksq[:], kf_p[:])
                qfT = []
                kfT = []
                for h in range(H_PER):
                    qt = hfeat.tile([NB, SBLK], bf16, name=f"qfT{blk}_{h}", tag="qfT")
                    nc.scalar.activation(qt[:], qsq[32 * h:32 * h + NB, :],
                                         mybir.ActivationFunctionType.Exp, scale=-0.5)
                    qfT.append(qt)
                    # 32-row padded so transposes below cover full 32-bands
                    kt = hfeat.tile([32, SBLK], bf16, name=f"kfT{blk}_{h}", tag="kfT")
                    nc.vector.memset(kt[:], 0.0)
                    nc.scalar.activation(kt[0:NB, :], ksq[32 * h:32 * h + NB, :],
                                         mybir.ActivationFunctionType.Exp, scale=-0.5)
                    kfT.append(kt)

                # ---- k features seq-major + normalizer ----
                kfp2 = psbig.tile([128, NSUB, 128], bf16, name=f"kfp2{blk}", tag="big")
                for j in range(NSUB):
                    for h in range(H_PER):
                        nc.tensor.transpose(kfp2[:, j, 32 * h:32 * h + 32],
                                            kfT[h][0:32, ts(j, 128)], ident[0:32, 0:32])
                kfu = featpool.tile([128, NSUB, 128], bf16, name=f"kfu{blk}", tag="kfu")
                nc.vector.tensor_copy(kfu[:], kfp2[:])
                ksum = miscpool.tile([128, NSUB * H_PER], f32, name=f"ksum{blk}", tag="ksum")
                nc.vector.reduce_sum(
                    ksum.rearrange("p (j g) -> p j g", g=H_PER),
                    kfu.rearrange("p j (g n) -> p j g n", n=32)[:, :, :, 0:NB],
                    axis=mybir.AxisListType.X)
                nc.vector.tensor_scalar_add(ksum[:], ksum[:], EPS)
                krec = miscpool.tile([128, NSUB * H_PER], f32, name=f"krec{blk}", tag="krec")
                nc.vector.reciprocal_approx_fast(krec[:], ksum[:])
                kfn = featpool.tile([128, NSUB, 128], bf16, name=f"kfn{blk}", tag="kfn")
                for j in range(NSUB):
                    for h in range(H_PER):
                        nc.vector.tensor_scalar_mul(
                            kfn[:, j, 32 * h:32 * h + 32],
                            kfu[:, j, 32 * h:32 * h + 32],
                            krec[:, 4 * j + h:4 * j + h + 1])

                # ---- causal scan, 128-chunks ----
                numT = []
                for h in range(H_PER):
                    numT.append(psnum.tile([128, SBLK], f32,
                                           name=f"numT{blk}_{h}", tag=f"numT{h}"))
                for j in range(NSUB):
                    at4 = psbig.tile([128, H_PER, 128], f32,
                                     name=f"at{blk}_{j}", tag="big")
                    for h in range(H_PER):
                        nc.tensor.matmul(at4[:, h, :],
                                         kfT[h][0:NB, ts(j, 128)],
                                         qfT[h][:, ts(j, 128)],
                                         start=True, stop=True)
                    atm4 = atmpool.tile([128, H_PER, 128], bf16,
                                        name=f"atm{blk}_{j}", tag="atm")
                    for h in range(H_PER):
                        nc.vector.scalar_tensor_tensor(
                            out=atm4[:, h, :], in0=at4[:, h, :],
                            scalar=krec[:, 4 * j + h:4 * j + h + 1],
                            in1=mask_sb[:],
                            op0=mybir.AluOpType.mult, op1=mybir.AluOpType.mult)
                    for h in range(H_PER):
                        nc.tensor.matmul(numT[h][:, ts(j, 128)],
                                         vha[:, j, h, 0:128], atm4[:, h, :],
                                         start=True, stop=False)
                        nc.tensor.matmul(numT[h][:, ts(j, 128)],
                                         Zb16[h][:, 0:128],
                                         qfT[h][:, ts(j, 128)],
                                         start=False, stop=True)
                    # state update (reads of Zb16 above precede these writes)
                    # bank-pitched (512 f32) so 32h-partition slices stay bank-aligned
                    su_p = psbig.tile([128, 512], f32, name=f"su{blk}_{j}", tag="big")
                    for h in range(H_PER):
                        nc.tensor.matmul(su_p[32 * h:32 * h + 32, 0:129],
                                         kfn[:, j, 32 * h:32 * h + 32],
                                         vha[:, j, h, 0:129],
                                         start=True, stop=True,
                                         tile_position=(0, 32 * h))
                    for h in range(H_PER):
                        nc.vector.tensor_add(Zb16[h][:, 0:129], Zb16[h][:, 0:129],
                                             su_p[32 * h:32 * h + NB, 0:129])

                # ---- denominator via K-cumsum (all heads at once) ----
                kcum = psbig.tile([128, SBLK], f32, name=f"kcum{blk}", tag="big")
                for j in range(NSUB):
                    nc.tensor.matmul(kcum[:, 128 * j:SBLK],
                                     kfn[:, j, :], maskb_sb[:, 0:SBLK - 128 * j],
                                     start=(j == 0), stop=(j == NSUB - 1))
                prod = featpool.tile([128, SBLK], bf16, name=f"prod{blk}", tag="prod")
                nc.vector.memset(prod[:], 0.0)
                for h in range(H_PER):
                    nc.vector.scalar_tensor_tensor(
                        out=prod[32 * h:32 * h + NB, :],
                        in0=kcum[32 * h:32 * h + NB, :],
                        scalar=zcol[h][:, 0:1], in1=qfT[h][:],
                        op0=mybir.AluOpType.add, op1=mybir.AluOpType.mult)
                for h in range(H_PER):
                    nc.vector.tensor_add(zcol[h][:, 0:1], zcol[h][:, 0:1],
                                         kcum[32 * h:32 * h + NB, SBLK - 1:SBLK])
                den_p = psbig.tile([128, SBLK], f32, name=f"den{blk}", tag="big")
                nc.tensor.matmul(den_p[:], bd_sb[:], prod[:], start=True, stop=True)
                drr = miscpool.tile([97, SBLK], f32, name=f"drr{blk}", tag="drr")
                nc.vector.tensor_scalar_add(drr[:], den_p[0:97, :], EPS)
                drr2 = miscpool.tile([97, SBLK], f32, name=f"drr2{blk}", tag="drr2")
                nc.vector.reciprocal_approx_fast(drr2[:], drr[:])
                drb = miscpool.tile([97, SBLK], bf16, name=f"drb{blk}", tag="drb")
                nc.vector.tensor_copy(drb[:], drr2[:])

                outT = []
                for h in range(H_PER):
                    bc_p = psbig.tile([128, SBLK], f32, name=f"bcp{blk}_{h}", tag="big")
                    nc.tensor.matmul(bc_p[:], ones_row[32 * h:32 * h + 1, :],
                                     drb[32 * h:32 * h + 1, :],
                                     start=True, stop=True,
                                     tile_position=(32 * h, 0))
                    numc = miscpool.tile([128, SBLK], bf16, name=f"numc{blk}_{h}", tag="numc")
                    nc.scalar.copy(numc[:], numT[h][:])
                    oT = otpool.tile([128, SBLK], bf16, name=f"oT{blk}_{h}", tag=f"outT{h}")
                    nc.vector.tensor_mul(oT[:], bc_p[:], numc[:])
                    outT.append(oT)

                # ---- output projection ----
                for j in range(NSUB):
                    osb = osbpool.tile([128, D], bf16, name=f"osb{blk}_{j}", tag="osb")
                    for c in range(4):
                        op = psbig.tile([128, 512], f32, name=f"op{blk}_{j}_{c}", tag="big")
                        for h in range(H_PER):
                            nc.tensor.matmul(op[:], outT[h][:, ts(j, 128)],
                                             wo_sb[:, h, ts(c, 512)],
                                             start=(h == 0), stop=(h == H_PER - 1))
                        if c != 3:
                            nc.scalar.copy(osb[:, ts(c, 512)], op[:])
                        else:
                            nc.vector.tensor_copy(osb[:, ts(c, 512)], op[:])
                    r0 = s0 + 128 * j
                    nc.sync.dma_start(part_d[r0:r0 + 128, :], osb[:])

    nc.compile()
    return nc


def _pad_feat(w):
    """[4, 8, D] head-feature weights -> [D, 128] with head h at rows 32h."""
    out = np.zeros((128, D), np.float32)
    for h in range(H_PER):
        out[32 * h:32 * h + NB] = w[h]
    return np.ascontiguousarray(out.T)


def _prep_inputs(q, k, v, w_q, w_k, w_v, w_o, omega):
    """Host-side sharding: returns in_maps for the 8 cores."""
    bf = ml_dtypes.bfloat16
    tri = np.triu(np.ones((128, 128), np.float32))   # mask[t, s] = t <= s
    mask = tri
    maskb = np.ones((128, 512), np.float32)
    maskb[:, :128] = tri
    bd = np.zeros((128, 128), np.float32)
    for h in range(H_PER):
        bd[32 * h:32 * h + NB, 32 * h] = 1.0

    xs = []
    for b in range(B):
        xs.append((np.ascontiguousarray(q[b].T).astype(bf),
                   np.ascontiguousarray(k[b].T).astype(bf),
                   np.ascontiguousarray(v[b].T).astype(bf)))

    # fused feature projections: per head, omega @ Wq_head  -> [8, 2048]
    wq_h = w_q.reshape(16, DK, D)                 # [head, dk, d_in]
    wk_h = w_k.reshape(16, DK, D)
    wqom = np.einsum('nd,hde->hne', omega, wq_h)  # [16, 8, D]
    wkom = np.einsum('nd,hde->hne', omega, wk_h)

    in_maps = []
    for core in range(8):
        b, g = divmod(core, 4)
        sl = slice(512 * g, 512 * (g + 1))
        hsl = slice(4 * g, 4 * (g + 1))
        xq, xk, xv = xs[b]
        in_maps.append({
            "xq": xq, "xk": xk, "xv": xv,
            "wqom": _pad_feat(wqom[hsl]).astype(bf),
            "wkom": _pad_feat(wkom[hsl]).astype(bf),
            "wv": np.ascontiguousarray(w_v[sl, :].T).astype(bf),
            "wo": np.ascontiguousarray(w_o[:, sl].T).astype(bf),
            "mask": mask,
            "maskb": maskb.astype(bf),
            "bd": bd.astype(bf),
        })
    return in_maps


def kernel(q, k, v, w_q, w_k, w_v, w_o, omega):
    global LAST_EXEC_TIME_NS
    q, k, v = np.asarray(q), np.asarray(k), np.asarray(v)
    w_q, w_k, w_v, w_o = (np.asarray(a) for a in (w_q, w_k, w_v, w_o))
    omega = np.asarray(omega)

    if "nc" not in _CACHE:
        _CACHE["nc"] = _build()
    nc = _CACHE["nc"]

    in_maps = _prep_inputs(q, k, v, w_q, w_k, w_v, w_o, omega)
    trace = bool(os.environ.get("BASS_KERNEL_TRACE"))
    res = run_bass_kernel_spmd(nc, in_maps, core_ids=list(range(8)), trace=trace)
    LAST_EXEC_TIME_NS = res.exec_time_ns

    out = np.zeros((B, S, D), np.float32)
    for core in range(8):
        b = core // 4
        out[b] += res.results[core]["part"].astype(np.float32)
    return out


# revision 33
# speedup vs baseline: 1.2939x; 1.0525x over previous
"""Causal Performer (FAVOR+) Trainium2 kernel, v2.

Sharding: 8 cores = 2 (batch) x 4 (head groups of 4 heads).  Each core
computes its 4 heads for one batch and returns a partial [4096, 2048]
output (its heads' contribution through w_o); the host sums the 4
partials per batch.

Key moves vs v1:
  - q/k head projections fused with the random-feature map on the host
    (qf = q @ (omega @ Wq_h).T), so on-chip contraction produces 8
    features per head (32 per core, padded to 128 rows at 32h offsets).
  - Causal scan chunked at 128 (not 512): per chunk, per head, one
    masked A^T matmul [128x128] + one intra numT matmul + one state
    matmul; state (Z | z) updated per chunk via one su matmul per head.
  - Denominator via a K-cumsum matmul chain shared by all 4 heads
    (stationary = seq-major normalized k features [128, 128], moving =
    causal mask) + block-diagonal reduction, instead of per-head
    ones-row matmul chains.
  - exp/square run once per tensor per block on [128, 512].
  - reciprocal_approx_fast for all reciprocals (values are >= eps).
  - x tiles double-buffered across blocks to keep the PE dense (HAM).
  - Output partials written bf16 (halves output DMA).

All matmuls bf16 with fp32 PSUM accumulation.
"""

import os
import numpy as np
import ml_dtypes

from concourse import bacc, mybir
import concourse.tile as tile
from concourse.bass import ts
from concourse.bass_utils import run_bass_kernel_spmd
from concourse.masks import make_identity

B, S, D = 2, 4096, 2048
H_PER = 4            # heads per core
DK = 128
NB = 8
SBLK = 512           # sequence block
NBLK = S // SBLK     # 8
NSUB = SBLK // 128   # 4 sub-chunks of 128
EPS = 1e-6

bf16 = mybir.dt.bfloat16
f32 = mybir.dt.float32

LAST_EXEC_TIME_NS = None
_CACHE = {}


def _build():
    nc = bacc.Bacc("TRN2", target_bir_lowering=False, debug=False)

    xq_d = nc.dram_tensor("xq", [D, S], bf16, kind="ExternalInput").ap()
    xk_d = nc.dram_tensor("xk", [D, S], bf16, kind="ExternalInput").ap()
    xv_d = nc.dram_tensor("xv", [D, S], bf16, kind="ExternalInput").ap()
    wqom_d = nc.dram_tensor("wqom", [D, 128], bf16, kind="ExternalInput").ap()
    wkom_d = nc.dram_tensor("wkom", [D, 128], bf16, kind="ExternalInput").ap()
    wv_d = nc.dram_tensor("wv", [D, 512], bf16, kind="ExternalInput").ap()
    wo_d = nc.dram_tensor("wo", [512, D], bf16, kind="ExternalInput").ap()
    mask_d = nc.dram_tensor("mask", [128, 128], f32, kind="ExternalInput").ap()
    maskb_d = nc.dram_tensor("maskb", [128, 512], bf16, kind="ExternalInput").ap()
    bd_d = nc.dram_tensor("bd", [128, 128], bf16, kind="ExternalInput").ap()
    part_d = nc.dram_tensor("part", [S, D], bf16, kind="ExternalOutput").ap()

    KC = D // 128    # 16 contraction chunks

    with tile.TileContext(nc) as tc:
        with tc.tile_pool(name="const", bufs=1) as const, \
             tc.tile_pool(name="wpool", bufs=1) as wpool, \
             tc.tile_pool(name="state", bufs=1) as state, \
             tc.tile_pool(name="xpool", bufs=2) as xpool, \
             tc.tile_pool(name="vpool", bufs=2) as vpool, \
             tc.tile_pool(name="featpool", bufs=2) as featpool, \
             tc.tile_pool(name="hfeat", bufs=8) as hfeat, \
             tc.tile_pool(name="atmpool", bufs=3) as atmpool, \
             tc.tile_pool(name="otpool", bufs=2) as otpool, \
             tc.tile_pool(name="osbpool", bufs=2) as osbpool, \
             tc.tile_pool(name="miscpool", bufs=2) as miscpool, \
             tc.tile_pool(name="psbig", bufs=2, space="PSUM") as psbig, \
             tc.tile_pool(name="psv", bufs=2, space="PSUM") as psv, \
             tc.tile_pool(name="psnum", bufs=1, space="PSUM") as psnum:

            ident = const.tile([128, 128], bf16, name="ident")
            make_identity(nc, ident)
            mask_sb = const.tile([128, 128], f32, name="mask_sb")
            nc.sync.dma_start(mask_sb[:], mask_d[:])
            maskb_sb = const.tile([128, 512], bf16, name="maskb_sb")
            nc.sync.dma_start(maskb_sb[:], maskb_d[:])
            bd_sb = const.tile([128, 128], bf16, name="bd_sb")
            nc.sync.dma_start(bd_sb[:], bd_d[:])
            ones_row = const.tile([97, 128], bf16, name="ones_row")
            nc.vector.memset(ones_row[:], 1.0)

            wv_sb = wpool.tile([128, KC, 512], bf16, name="wv_sb")
            wvr = wv_d.rearrange("(c p) m -> p c m", p=128)
            nc.sync.dma_start(wv_sb[:, 0:KC // 2, :], wvr[:, 0:KC // 2, :])
            nc.sync.dma_start(wv_sb[:, KC // 2:KC, :], wvr[:, KC // 2:KC, :])
            wqom_sb = wpool.tile([128, KC, 128], bf16, name="wqom_sb")
            nc.sync.dma_start(wqom_sb[:], wqom_d.rearrange("(c p) m -> p c m", p=128))
            wkom_sb = wpool.tile([128, KC, 128], bf16, name="wkom_sb")
            nc.sync.dma_start(wkom_sb[:], wkom_d.rearrange("(c p) m -> p c m", p=128))
            wo_sb = wpool.tile([128, H_PER, D], bf16, name="wo_sb")
            nc.sync.dma_start(wo_sb[:], wo_d.rearrange("(c p) m -> p c m", p=128))

            # persistent scan state: Zsb rows at 32h (8 valid per head),
            # cols 0:128 = Z, col 128 = z (k' mass); zcol for K-cumsum.
            Zb16 = []
            for h in range(H_PER):
                zb = state.tile([NB, 132], bf16, name=f"Zb16_{h}")
                nc.vector.memset(zb[:], 0.0)
                Zb16.append(zb)
            zcol = []
            for h in range(H_PER):
                zc = state.tile([NB, 1], f32, name=f"zcol{h}")
                nc.vector.memset(zc[:], 0.0)
                zcol.append(zc)

            for blk in range(NBLK):
                s0 = blk * SBLK

                xv_sb = xpool.tile([128, KC, SBLK], bf16, name=f"xv{blk}", tag="xv")
                xq_sb = xpool.tile([128, KC, SBLK], bf16, name=f"xq{blk}", tag="xq")
                xk_sb = xpool.tile([128, KC, SBLK], bf16, name=f"xk{blk}", tag="xk")
                for xsb, xd in ((xv_sb, xv_d), (xq_sb, xq_d), (xk_sb, xk_d)):
                    xr = xd.rearrange("(c p) s -> p c s", p=128)
                    nc.gpsimd.dma_start(xsb[:, 0:KC // 2, :],
                                        xr[:, 0:KC // 2, s0:s0 + SBLK])
                    nc.gpsimd.dma_start(xsb[:, KC // 2:KC, :],
                                        xr[:, KC // 2:KC, s0:s0 + SBLK])

                # ---- v projection: vha [s_sub(128), j, head, 132] (+ones col) ----
                vha = vpool.tile([128, NSUB, H_PER, 132], bf16, name=f"vha{blk}", tag="vha")
                for j in range(NSUB):
                    pp = psv.tile([128, SBLK], f32, name=f"pv{blk}_{j}", tag="pv")
                    for kc in range(KC):
                        nc.tensor.matmul(pp[:], xv_sb[:, kc, ts(j, 128)],
                                         wv_sb[:, kc, :],
                                         start=(kc == 0), stop=(kc == KC - 1))
                    nc.scalar.copy(vha[:, j, :, 0:128],
                                   pp.rearrange("p (h d) -> p h d", d=128))
                    nc.vector.memset(vha[:, j, :, 128:129], 1.0)

                # ---- fused q/k feature projections -> [128(pad), 512] ----
                qf_p = psbig.tile([128, SBLK], f32, name=f"qfp{blk}", tag="big")
                kf_p = psbig.tile([128, SBLK], f32, name=f"kfp{blk}", tag="big")
                for dst, wsb, xsb in ((qf_p, wqom_sb, xq_sb), (kf_p, wkom_sb, xk_sb)):
                    for kc in range(KC):
                        nc.tensor.matmul(dst[:], wsb[:, kc, :], xsb[:, kc, :],
                                         start=(kc == 0), stop=(kc == KC - 1))
                qsq = miscpool.tile([128, SBLK], f32, name=f"qsq{blk}", tag="sq")
                nc.scalar.square(qsq[:], qf_p[:])
                ksq = miscpool.tile([128, SBLK], f32, name=f"ksq{blk}", tag="sq")
                nc.scalar.square(ksq[:], kf_p[:])
                qfT = []
                kfT = []
                for h in range(H_PER):
                    qt = hfeat.tile([NB, SBLK], bf16, name=f"qfT{blk}_{h}", tag="qfT")
                    nc.scalar.activation(qt[:], qsq[32 * h:32 * h + NB, :],
                                         mybir.ActivationFunctionType.Exp, scale=-0.5)
                    qfT.append(qt)
                    # 32-row padded so transposes below cover full 32-bands
                    kt = hfeat.tile([32, SBLK], bf16, name=f"kfT{blk}_{h}", tag="kfT")
                    nc.vector.memset(kt[:], 0.0)
                    nc.scalar.activation(kt[0:NB, :], ksq[32 * h:32 * h + NB, :],
                                         mybir.ActivationFunctionType.Exp, scale=-0.5)
                    kfT.append(kt)

                # ---- k features seq-major + normalizer ----
                kfp2 = psbig.tile([128, NSUB, 128], bf16, name=f"kfp2{blk}", tag="big")
                for j in range(NSUB):
                    for h in range(H_PER):
                        nc.tensor.transpose(kfp2[:, j, 32 * h:32 * h + 32],
                                            kfT[h][0:32, ts(j, 128)], ident[0:32, 0:32])
                kfu = featpool.tile([128, NSUB, 128], bf16, name=f"kfu{blk}", tag="kfu")
                nc.vector.tensor_copy(kfu[:], kfp2[:])
                ksum = miscpool.tile([128, NSUB * H_PER], f32, name=f"ksum{blk}", tag="ksum")
                nc.vector.reduce_sum(
                    ksum.rearrange("p (j g) -> p j g", g=H_PER),
                    kfu.rearrange("p j (g n) -> p j g n", n=32)[:, :, :, 0:NB],
                    axis=mybir.AxisListType.X)
                nc.vector.tensor_scalar_add(ksum[:], ksum[:], EPS)
                krec = miscpool.tile([128, NSUB * H_PER], f32, name=f"krec{blk}", tag="krec")
                nc.vector.reciprocal_approx_fast(krec[:], ksum[:])
                kfn = featpool.tile([128, NSUB, 128], bf16, name=f"kfn{blk}", tag="kfn")
                nc.vector.tensor_tensor(
                    out=kfn.rearrange("p j (g n) -> p j g n", n=32),
                    in0=kfu.rearrange("p j (g n) -> p j g n", n=32),
                    in1=krec.rearrange("p (j g) -> p j g", g=H_PER)[:, :, :, None]
                        .to_broadcast([128, NSUB, H_PER, 32]),
                    op=mybir.AluOpType.mult)

                # ---- causal scan, 128-chunks ----
                numT = []
                for h in range(H_PER):
                    numT.append(psnum.tile([128, SBLK], f32,
                                           name=f"numT{blk}_{h}", tag=f"numT{h}"))
                for j in range(NSUB):
                    at4 = psbig.tile([128, H_PER, 128], f32,
                                     name=f"at{blk}_{j}", tag="big")
                    for h in range(H_PER):
                        nc.tensor.matmul(at4[:, h, :],
                                         kfT[h][0:NB, ts(j, 128)],
                                         qfT[h][:, ts(j, 128)],
                                         start=True, stop=True)
                    atm4 = atmpool.tile([128, H_PER, 128], bf16,
                                        name=f"atm{blk}_{j}", tag="atm")
                    for h in range(H_PER):
                        nc.vector.scalar_tensor_tensor(
                            out=atm4[:, h, :], in0=at4[:, h, :],
                            scalar=krec[:, 4 * j + h:4 * j + h + 1],
                            in1=mask_sb[:],
                            op0=mybir.AluOpType.mult, op1=mybir.AluOpType.mult)
                    for h in range(H_PER):
                        nc.tensor.matmul(numT[h][:, ts(j, 128)],
                                         vha[:, j, h, 0:128], atm4[:, h, :],
                                         start=True, stop=False)
                        nc.tensor.matmul(numT[h][:, ts(j, 128)],
                                         Zb16[h][:, 0:128],
                                         qfT[h][:, ts(j, 128)],
                                         start=False, stop=True)
                    # state update (reads of Zb16 above precede these writes)
                    # bank-pitched (512 f32) so 32h-partition slices stay bank-aligned
                    su_p = psbig.tile([128, 512], f32, name=f"su{blk}_{j}", tag="big")
                    for h in range(H_PER):
                        nc.tensor.matmul(su_p[32 * h:32 * h + 32, 0:129],
                                         kfn[:, j, 32 * h:32 * h + 32],
                                         vha[:, j, h, 0:129],
                                         start=True, stop=True,
                                         tile_position=(0, 32 * h))
                    for h in range(H_PER):
                        nc.vector.tensor_add(Zb16[h][:, 0:129], Zb16[h][:, 0:129],
                                             su_p[32 * h:32 * h + NB, 0:129])

                # ---- denominator via K-cumsum (all heads at once) ----
                kcum = psbig.tile([128, SBLK], f32, name=f"kcum{blk}", tag="big")
                for j in range(NSUB):
                    nc.tensor.matmul(kcum[:, 128 * j:SBLK],
                                     kfn[:, j, :], maskb_sb[:, 0:SBLK - 128 * j],
                                     start=(j == 0), stop=(j == NSUB - 1))
                prod = featpool.tile([128, SBLK], bf16, name=f"prod{blk}", tag="prod")
                nc.vector.memset(prod[:], 0.0)
                for h in range(H_PER):
                    nc.vector.scalar_tensor_tensor(
                        out=prod[32 * h:32 * h + NB, :],
                        in0=kcum[32 * h:32 * h + NB, :],
                        scalar=zcol[h][:, 0:1], in1=qfT[h][:],
                        op0=mybir.AluOpType.add, op1=mybir.AluOpType.mult)
                for h in range(H_PER):
                    nc.vector.tensor_add(zcol[h][:, 0:1], zcol[h][:, 0:1],
                                         kcum[32 * h:32 * h + NB, SBLK - 1:SBLK])
                den_p = psbig.tile([128, SBLK], f32, name=f"den{blk}", tag="big")
                nc.tensor.matmul(den_p[:], bd_sb[:], prod[:], start=True, stop=True)
                drr = miscpool.tile([97, SBLK], f32, name=f"drr{blk}", tag="drr")
                nc.vector.tensor_scalar_add(drr[:], den_p[0:97, :], EPS)
                drr2 = miscpool.tile([97, SBLK], f32, name=f"drr2{blk}", tag="drr2")
                nc.vector.reciprocal_approx_fast(drr2[:], drr[:])
                drb = miscpool.tile([97, SBLK], bf16, name=f"drb{blk}", tag="drb")
                nc.vector.tensor_copy(drb[:], drr2[:])

                outT = []
                for h in range(H_PER):
                    bc_p = psbig.tile([128, SBLK], f32, name=f"bcp{blk}_{h}", tag="big")
                    nc.tensor.matmul(bc_p[:], ones_row[32 * h:32 * h + 1, :],
                                     drb[32 * h:32 * h + 1, :],
                                     start=True, stop=True,
                                     tile_position=(32 * h, 0))
                    numc = miscpool.tile([128, SBLK], bf16, name=f"numc{blk}_{h}", tag="numc")
                    nc.scalar.copy(numc[:], numT[h][:])
                    oT = otpool.tile([128, SBLK], bf16, name=f"oT{blk}_{h}", tag=f"outT{h}")
                    nc.vector.tensor_mul(oT[:], bc_p[:], numc[:])
                    outT.append(oT)

                # ---- output projection ----
                for j in range(NSUB):
                    osb = osbpool.tile([128, D], bf16, name=f"osb{blk}_{j}", tag="osb")
                    for c in range(4):
                        op = psbig.tile([128, 512], f32, name=f"op{blk}_{j}_{c}", tag="big")
                        for h in range(H_PER):
                            nc.tensor.matmul(op[:], outT[h][:, ts(j, 128)],
                                             wo_sb[:, h, ts(c, 512)],
                                             start=(h == 0), stop=(h == H_PER - 1))
                        if c != 3:
                            nc.scalar.copy(osb[:, ts(c, 512)], op[:])
                        else:
                            nc.vector.tensor_copy(osb[:, ts(c, 512)], op[:])
                    r0 = s0 + 128 * j
                    nc.sync.dma_start(part_d[r0:r0 + 128, :], osb[:])

    nc.compile()
    return nc


def _pad_feat(w):
    """[4, 8, D] head-feature weights -> [D, 128] with head h at rows 32h."""
    out = np.zeros((128, D), np.float32)
    for h in range(H_PER):
        out[32 * h:32 * h + NB] = w[h]
    return np.ascontiguousarray(out.T)


def _prep_inputs(q, k, v, w_q, w_k, w_v, w_o, omega):
    """Host-side sharding: returns in_maps for the 8 cores."""
    bf = ml_dtypes.bfloat16
    tri = np.triu(np.ones((128, 128), np.float32))   # mask[t, s] = t <= s
    mask = tri
    maskb = np.ones((128, 512), np.float32)
    maskb[:, :128] = tri
    bd = np.zeros((128, 128), np.float32)
    for h in range(H_PER):
        bd[32 * h:32 * h + NB, 32 * h] = 1.0

    xs = []
    for b in range(B):
        xs.append((np.ascontiguousarray(q[b].T).astype(bf),
                   np.ascontiguousarray(k[b].T).astype(bf),
                   np.ascontiguousarray(v[b].T).astype(bf)))

    # fused feature projections: per head, omega @ Wq_head  -> [8, 2048]
    wq_h = w_q.reshape(16, DK, D)                 # [head, dk, d_in]
    wk_h = w_k.reshape(16, DK, D)
    wqom = np.einsum('nd,hde->hne', omega, wq_h)  # [16, 8, D]
    wkom = np.einsum('nd,hde->hne', omega, wk_h)

    in_maps = []
    for core in range(8):
        b, g = divmod(core, 4)
        sl = slice(512 * g, 512 * (g + 1))
        hsl = slice(4 * g, 4 * (g + 1))
        xq, xk, xv = xs[b]
        in_maps.append({
            "xq": xq, "xk": xk, "xv": xv,
            "wqom": _pad_feat(wqom[hsl]).astype(bf),
            "wkom": _pad_feat(wkom[hsl]).astype(bf),
            "wv": np.ascontiguousarray(w_v[sl, :].T).astype(bf),
            "wo": np.ascontiguousarray(w_o[:, sl].T).astype(bf),
            "mask": mask,
            "maskb": maskb.astype(bf),
            "bd": bd.astype(bf),
        })
    return in_maps


def kernel(q, k, v, w_q, w_k, w_v, w_o, omega):
    global LAST_EXEC_TIME_NS
    q, k, v = np.asarray(q), np.asarray(k), np.asarray(v)
    w_q, w_k, w_v, w_o = (np.asarray(a) for a in (w_q, w_k, w_v, w_o))
    omega = np.asarray(omega)

    if "nc" not in _CACHE:
        _CACHE["nc"] = _build()
    nc = _CACHE["nc"]

    in_maps = _prep_inputs(q, k, v, w_q, w_k, w_v, w_o, omega)
    trace = bool(os.environ.get("BASS_KERNEL_TRACE"))
    res = run_bass_kernel_spmd(nc, in_maps, core_ids=list(range(8)), trace=trace)
    LAST_EXEC_TIME_NS = res.exec_time_ns

    out = np.zeros((B, S, D), np.float32)
    for core in range(8):
        b = core // 4
        out[b] += res.results[core]["part"].astype(np.float32)
    return out
